# revision 27
# baseline (speedup 1.0000x reference)
"""Multi-head attention kernel for Trainium2 (Bass/Tile), 8-core SPMD.

Problem: qkv (4, 1536, 2048) fp32, NUM_HEADS=8, ch=64.
  q,k,v = split(qkv, 3, axis=1); scale=ch**-0.25
  w = softmax((q*s)^T (k*s)) per head; out = w @ v -> (4, 512, 2048)

Sharding: B*H = 32 head-instances, 4 contiguous heads per core (8 cores).
Per-core inputs: q/k/v blocks (256, 2048); output (256, 2048).

Per-head pipeline on one core (C=64, T=2048):
  mm1:  wT[s_tile(128), t] = k_sb[:, s_tile]^T-contract-c @ q_sb  (K=64)
  exp:  eT = Exp(0.125 * wT)  on ScalarE, PSUM -> SBUF   (bottleneck engine)
  mm2:  po[m, t] += vt_aug[s_tile]^T @ eT   (K=128, M=65)
        vt_aug cols 0:64 = v^T chunk, col 64 = ones -> po row 64 = Z[t]
  tail: Z -> SBUF -> DRAM roundtrip broadcast to 64 partitions,
        approx-reciprocal (~2ulp) at base 0, o = po[0:64] * rzb.

Matmuls run in float32r (TF32-class, 1 cycle/row vs fp32's 4): inputs are
declared float32r in DRAM so no conversion passes are needed; PSUM stays
fp32. mm_dtype="f32" gives the full-fp32 fallback.
"""

import numpy as np
from contextlib import ExitStack

B = 4
NUM_HEADS = 8
C = 64
T = 2048
N_CORES = 8
HPC = (B * NUM_HEADS) // N_CORES  # heads per core = 4
R = HPC * C  # 256 rows per core

MM_DTYPE = "f32r"

_NC_CACHE = {}


def build_nc(t=T, hpc=HPC, mm_dtype=MM_DTYPE, reps=1):
    import concourse.mybir as mybir
    import concourse.tile as tile
    from concourse import bacc

    f32 = mybir.dt.float32
    fmm = mybir.dt.float32r if mm_dtype == "f32r" else mybir.dt.float32
    Exp = mybir.ActivationFunctionType.Exp
    st = t // 128  # number of s tiles
    th_size = 1024 if t % 1024 == 0 else t  # t processed in halves for PSUM
    n_th = t // th_size
    chunk = min(512, th_size)  # matmul moving-operand max
    n_chunk = th_size // chunk
    scale = 1.0 / np.sqrt(C)

    nc = bacc.Bacc("TRN2", debug=False, num_devices=N_CORES)
    q_d = nc.dram_tensor("q", (hpc * C, t), fmm, kind="ExternalInput")
    k_d = nc.dram_tensor("k", (hpc * C, t), fmm, kind="ExternalInput")
    v_d = nc.dram_tensor("v", (hpc * C, t), f32, kind="ExternalInput")
    o_d = nc.dram_tensor("o", (hpc * C, t), f32, kind="ExternalOutput")

    with tile.TileContext(nc) as tc, ExitStack() as ctx:
        # v^T tiles (with ones column) for all heads. Built WITHOUT the PE
        # (no PSUM banks, no PE-queue blocking at kernel start): DVE 32x32
        # stream-transpose -> DRAM roundtrip -> 4 strided reassembly DMAs
        # (contiguous 128B runs) -> one converting DVE copy to f32r.
        vt_pool = ctx.enter_context(tc.tile_pool(name="vt", bufs=1))
        vt = [
            vt_pool.tile([128, st, 65], fmm, tag=f"vt{h}", name=f"vt{h}")
            for h in range(hpc)
        ]
        # Per-head load order: q/k for head h, then head h's v-transpose
        # chain — head 0's mm1 inputs land first, and vt(h0) (which gates
        # mm2 and thus et-slot recycling) isn't queued behind h1-h3 traffic.
        qk_pool = ctx.enter_context(tc.tile_pool(name="qk", bufs=hpc))
        vload = ctx.enter_context(tc.tile_pool(name="vload", bufs=2))
        vdram = ctx.enter_context(tc.tile_pool(name="vdram", bufs=2, space="DRAM"))
        q_sbs, k_sbs = {}, {}

        def emit_qk(h):
            q_sb = qk_pool.tile([64, t], fmm, tag="q", name=f"qsb{h}")
            k_sb = qk_pool.tile([64, t], fmm, tag="k", name=f"ksb{h}")
            nc.sync.dma_start(out=q_sb, in_=q_d[h * 64 : (h + 1) * 64, :])
            nc.sync.dma_start(out=k_sb, in_=k_d[h * 64 : (h + 1) * 64, :])
            q_sbs[h], k_sbs[h] = q_sb, k_sb

        def emit_vt(h):
            v_sb = vload.tile([64, t], f32, tag="v", name="vsb")
            nc.sync.dma_start(out=v_sb, in_=v_d[h * 64 : (h + 1) * 64, :])
            vts = vload.tile([64, t], f32, tag="vts", name="vts")
            nc.vector.transpose(out=vts, in_=v_sb)
            vtd = vdram.tile([64, t], f32, name="vtd")
            nc.sync.dma_start(out=vtd, in_=vts)
            vt_f32 = vload.tile([128, st, 65], f32, tag="vtf", name="vtf")
            src = vtd.rearrange(
                "(b i) (s four j) -> b i s four j", b=2, i=32, four=4, j=32
            )
            for a in range(4):
                for bb in range(2):
                    nc.sync.dma_start(
                        out=vt_f32[32 * a : 32 * (a + 1), :, 32 * bb : 32 * (bb + 1)],
                        in_=src[bb, :, :, a, :],
                    )
            nc.gpsimd.memset(vt_f32[:, :, 64:65], 1.0)
            nc.vector.tensor_copy(
                out=vt[h].rearrange("p s c -> p (s c)"),
                in_=vt_f32.rearrange("p s c -> p (s c)"),
            )

        emit_vt(0)
        for h in range(hpc):
            emit_qk(h)
        for h in range(1, hpc):
            emit_vt(h)

        et_pool = ctx.enter_context(tc.tile_pool(name="et", bufs=10))
        osb_pool = ctx.enter_context(tc.tile_pool(name="osb", bufs=2))
        rz_pool = ctx.enter_context(tc.tile_pool(name="rz", bufs=2))
        dram_pool = ctx.enter_context(tc.tile_pool(name="dscr", bufs=2, space="DRAM"))
        pw_pool = ctx.enter_context(tc.tile_pool(name="pw", bufs=2, space="PSUM"))
        po_pool = ctx.enter_context(tc.tile_pool(name="po", bufs=2, space="PSUM"))

        # Software-pipelined emission over flat (head, t-half, s) iterations:
        # mm1 for iteration i+1 is emitted BEFORE mm2 of iteration i so the
        # PE's in-order queue never parks mm1 behind an exp-blocked mm2 —
        # otherwise ScalarE (the bottleneck) starves every iteration.
        flat = [
            (rep, h, thi, s)
            for rep in range(reps)
            for h in range(hpc)
            for thi in range(n_th)
            for s in range(st)
        ]
        pw_tiles = {}

        def emit_mm1(it):
            rep, h, thi, s = it
            t0 = thi * th_size
            pw = pw_pool.tile([128, th_size], f32, name="pw")
            for cc in range(n_chunk):
                nc.tensor.matmul(
                    pw[:, cc * chunk : (cc + 1) * chunk],
                    k_sbs[h][:, s * 128 : (s + 1) * 128],
                    q_sbs[h][:, t0 + cc * chunk : t0 + (cc + 1) * chunk],
                    start=True,
                    stop=True,
                )
            pw_tiles[it] = pw

        emit_mm1(flat[0])
        po = None
        for i, it in enumerate(flat):
            rep, h, thi, s = it
            t0 = thi * th_size
            if i + 1 < len(flat):
                emit_mm1(flat[i + 1])
            pw = pw_tiles.pop(it)
            et = et_pool.tile([128, th_size], fmm, name="et")
            nc.scalar.activation(out=et, in_=pw, func=Exp, scale=scale)
            if s == 0:
                po = po_pool.tile([65, th_size], f32, name="po")
            for cc in range(n_chunk):
                nc.tensor.matmul(
                    po[:, cc * chunk : (cc + 1) * chunk],
                    vt[h][:, s, :],
                    et[:, cc * chunk : (cc + 1) * chunk],
                    start=(s == 0),
                    stop=(s == st - 1),
                )
            if s == st - 1:
                # normalization tail for this (head, t-half).
                # reciprocal_approx_* misbehaves on HW when reading PSUM or at
                # base partition != 0, so: copy Z row to SBUF, broadcast it to
                # 64 partitions via a DRAM roundtrip, then reciprocal at base 0.
                zcp = rz_pool.tile([65, th_size], f32, tag="zcp", name="zcp")
                nc.vector.tensor_copy(out=zcp[64:65, :], in_=po[64:65, :])
                zd = dram_pool.tile([1, th_size], f32, name="zd")
                nc.sync.dma_start(out=zd, in_=zcp[64:65, :])
                zb = rz_pool.tile([64, th_size], f32, tag="zb", name="zb")
                nc.sync.dma_start(out=zb, in_=zd.partition_broadcast(64))
                rzb = rz_pool.tile([64, th_size], f32, tag="rzb", name="rzb")
                nc.vector.reciprocal_approx_fast(out=rzb, in_=zb)
                o_sb = osb_pool.tile([64, th_size], f32, name="osb")
                nc.vector.tensor_mul(o_sb, po[0:64, :], rzb)
                nc.sync.dma_start(
                    out=o_d[h * 64 : (h + 1) * 64, t0 : t0 + th_size], in_=o_sb
                )

    nc.compile()
    return nc


def get_nc(t=T, hpc=HPC, mm_dtype=MM_DTYPE):
    key = (t, hpc, mm_dtype)
    if key not in _NC_CACHE:
        _NC_CACHE[key] = build_nc(t, hpc, mm_dtype)
    return _NC_CACHE[key]


def make_in_maps(qkv):
    """Slice the full qkv into per-core q/k/v blocks."""
    qkv = np.ascontiguousarray(qkv)
    in_maps = []
    for m in range(N_CORES):
        b = m // 2
        h0 = HPC * (m % 2)
        r0 = h0 * C
        in_maps.append(
            {
                "q": np.ascontiguousarray(qkv[b, r0 : r0 + R, :]),
                "k": np.ascontiguousarray(qkv[b, 512 + r0 : 512 + r0 + R, :]),
                "v": np.ascontiguousarray(qkv[b, 1024 + r0 : 1024 + r0 + R, :]),
            }
        )
    return in_maps


def assemble_out(results, qkv_shape):
    out = np.empty((B, NUM_HEADS * C, T), dtype=np.float32)
    for m in range(N_CORES):
        b = m // 2
        r0 = HPC * (m % 2) * C
        out[b, r0 : r0 + R, :] = results[m]["o"]
    return out


def kernel(qkv):
    from concourse.bass_utils import run_bass_kernel_spmd

    nc = get_nc()
    in_maps = make_in_maps(np.asarray(qkv, dtype=np.float32))
    res = run_bass_kernel_spmd(nc, in_maps, core_ids=list(range(N_CORES)))
    return assemble_out(res.results, qkv.shape)
